# revision 68
# baseline (speedup 1.0000x reference)
"""Trainium2 Bass kernel for nn_AttentionBlock (GroupNorm + 4-head attention + proj + residual).

Sharding: 8 cores = (batch b in 0..3) x (head-pair p in 0..1).  Each core gets
x[b] and the weight slices for its two heads, computes GroupNorm -> QKV ->
attention -> partial proj (over its 128 attention-output channels), and returns
a partial [256, 4096] projection output.  The host sums the two partials per
batch, adds proj bias + residual, and reshapes.

The device program is identical on all cores (SPMD); all per-core variation is
carried by the input tensors.

Layout notes (per core):
  h   = groupnorm(x[b])                SBUF [c=128 x2, n=4096] bf16
  Q2  = log2e * (Wq_pair^T . h + bq)   SBUF [128 (2 heads x 64 d), 4096] bf16
  K2  = Wk_pair^T . h + bk             SBUF [128, 4096] bf16
  V1  = [h^T . Wv_pair | ones]         SBUF [128 (m-chunk), 32, 2, 80] fp8e4
  scores'^T[m, n] = sum_d K[d,m] Q[d,n] via matmul(lhsT=K2[64 rows], rhs=Q2),
     two heads packed concurrently in PE row-groups (0,0) and (64,0);
     = log2e * raw scores thanks to the host-side Q pre-scale.
  pexp (fp8e4, ~exp(0.125*raw)*2^(-C/8)) is produced half-and-half:
     ScalarE: exact Exp with free affine (scale 0.125/log2e, bias -C/8*ln2)
     DVE:     Schraudolph bits8 = clamp(qk' + 56 - C) via one tensor_scalar
              (max, add) written through an int8 view of the fp8 tile.
     NB fp8e4 here is IEEE-style: exponent 15 = Inf/NaN, so C keeps bits <= ~116.
  pv[d+1, n] += V1^T . pexp            fp8 DoubleRow matmuls, each contracting
     a PAIR of 128-key chunks (lhsT [128,2,65], rhs [128,2,512]); row 64
     (the ones column) accumulates the softmax denominator.
  At the end of each n-block: den rows copied out on the DVE, raw A on
  ScalarE (frees the PSUM banks fast), then off the critical path:
  DRAM-bounce broadcast of den, approx-reciprocal on the [128,512] broadcast
  (base-partition 0 only - the custom-DVE op breaks at other bases), A *= 1/den,
  proj accumulates both heads into one PSUM tile per m.
  y_partial = Wp[:, pair]^T . A        [256, 4096] -> DRAM
"""

import numpy as np

import concourse.bacc as bacc
import concourse.bass as bass

import concourse.mybir as mybir
import concourse.tile as tile

B = 4
C = 256
N = 4096          # 64*64
NH = 4
D = 64            # head dim
GROUPS = 32
EPS = 1e-5
NCORES = 8
SCALE = float(D) ** -0.5  # 0.125
F32 = mybir.dt.float32
F32R = mybir.dt.float32r
BF16 = mybir.dt.bfloat16
F8 = mybir.dt.float8e4
I8 = mybir.dt.int8

MB = 32           # m chunks of 128
NB = 8            # n chunks of 512

# fp8-e4m3 softmax weights + DoubleRow PV.
#
# The Q weights/bias are pre-scaled by log2(e) on the host, so the QK matmul
# emits qk' = log2e * qk_raw, i.e. 8*t where exp(0.125*qk_raw) = 2^t.
#  - ScalarE half (head-lo): exact exp -> fp8e4 out, scaled by 2^(-C/8) via
#    the activation's free affine (scale = 0.125/log2e, bias = -C/8*ln2).
#  - DVE half (head-hi): Schraudolph bits8 = clamp(qk' + 56 - C, >=0) via one
#    tensor_scalar (op0 = max, op1 = add) written through an int8 view.
# The 2^(-C/8) factor cancels in the softmax normalization.  PV runs fp8
# DoubleRow: each matmul contracts TWO 128-key chunks at once (rhs [128,2,512],
# lhsT [128,2,65]), halving the PV instruction count.
LOG2E = 1.4426950408889634
LN2 = 0.6931471805599453
F8C = 24.0                      # exponent offset: bits8 <= ~116 (qk' max ~84); fp8e4 here is IEEE-style, exponent 15 (bits >= 120) = Inf/NaN
EXPSC = 0.125 / LOG2E           # ACT exp input scale (qk' -> 0.125*qk_raw)
EXPB = -(F8C / 8.0) * LN2       # ACT exp input bias  (output *= 2^(-C/8))
SCHROFF = 56.0 - F8C            # DVE: bits8 = max(qk', -SCHROFF) + SCHROFF


def _build_program(has_v_bias: bool, attn_dtype=BF16, mm_dtype=BF16, chain: int = 1):
    nc = bacc.Bacc("TRN2", target_bir_lowering=False)
    MMD = mm_dtype        # dtype of tiles feeding qkv/proj matmuls (h, weights, A)
    MAD = attn_dtype      # dtype of lhsT tiles for QK/PV (K2, V1)
    MADR = attn_dtype     # dtype of rhs tiles for QK/PV (Q2, pexp)

    xb = nc.dram_tensor("xb", [C, N], F32, kind="ExternalInput")
    wqkT = nc.dram_tensor("wqkT", [C, 256], F32, kind="ExternalInput")   # cols 0:128 Qpair, 128:256 Kpair
    wvT = nc.dram_tensor("wvT", [C, 128], F32, kind="ExternalInput")
    wpT = nc.dram_tensor("wpT", [128, C], F32, kind="ExternalInput")     # rows = pair channels
    gnw = nc.dram_tensor("gnw", [C], F32, kind="ExternalInput")
    gnb = nc.dram_tensor("gnb", [C], F32, kind="ExternalInput")
    qkb = nc.dram_tensor("qkb", [256], F32, kind="ExternalInput")        # 0:128 Q bias pair, 128:256 K bias pair
    if has_v_bias:
        vb = nc.dram_tensor("vb", [128], F32, kind="ExternalInput")
    yp = nc.dram_tensor("yp", [C, N], F32, kind="ExternalOutput")

    # group-indicator matrix: G[c, g] = 1 if c // 8 == g else 0  (per c-tile)
    g_host = np.zeros((128, 16), dtype=np.float32)
    for c in range(128):
        g_host[c, c // 8] = 1.0
    g_dram = nc.inline_tensor(g_host, name="gmat")
    gt_dram = nc.inline_tensor(np.ascontiguousarray(g_host.T), name="gmatT")

    with tile.TileContext(nc) as tc:
        with (
            tc.tile_pool(name="consts", bufs=1) as consts,
            tc.tile_pool(name="xh", bufs=2) as xh_pool,
            tc.tile_pool(name="hp", bufs=2) as hp_pool,
            tc.tile_pool(name="qk_sb", bufs=1) as qk_sb,
            tc.tile_pool(name="v1p", bufs=1) as v1p,
            tc.tile_pool(name="st", bufs=2) as st,
            tc.tile_pool(name="pexpa", bufs=6) as pexpa,
            tc.tile_pool(name="denp", bufs=4) as denp,
            tc.tile_pool(name="bcp", bufs=8) as bcp,
            tc.tile_pool(name="drec", bufs=4, space="DRAM") as drec,
            tc.tile_pool(name="yout", bufs=4) as yout,
        ):
            # ---- constants ----
            g_sb = consts.tile([128, 16], F32, tag="gsb")
            nc.sync.dma_start(out=g_sb, in_=g_dram[:, :])
            gt_sb = consts.tile([16, 128], F32, tag="gtsb")
            nc.sync.dma_start(out=gt_sb, in_=gt_dram[:, :])

            def load_weight(shape, dram_slice, tag):
                w_raw = consts.tile(shape, F32, tag=tag + "_r", name=tag + "_r")
                nc.sync.dma_start(out=w_raw, in_=dram_slice)
                if MMD == F32:
                    return w_raw
                w_c = consts.tile(shape, MMD, tag=tag, name=tag)
                nc.vector.tensor_copy(w_c, w_raw)
                return w_c

            w_qk = [load_weight([128, 256], wqkT[t * 128:(t + 1) * 128, :], f"wqk{t}")
                    for t in range(2)]
            w_v = [load_weight([128, 128], wvT[t * 128:(t + 1) * 128, :], f"wv{t}")
                   for t in range(2)]
            wp_lo = load_weight([64, 256], wpT[0:64, :], "wplo")
            wp_hi = load_weight([64, 256], wpT[64:128, :], "wphi")

            gw_t, gb_t = [], []
            for t in range(2):
                gwt = consts.tile([128, 1], F32, tag=f"gw{t}")
                nc.sync.dma_start(out=gwt, in_=gnw[t * 128:(t + 1) * 128])
                gw_t.append(gwt)
                gbt = consts.tile([128, 1], F32, tag=f"gb{t}")
                nc.sync.dma_start(out=gbt, in_=gnb[t * 128:(t + 1) * 128])
                gb_t.append(gbt)
            bq = consts.tile([128, 1], F32, tag="bq")
            nc.sync.dma_start(out=bq, in_=qkb[0:128])
            bk = consts.tile([128, 1], F32, tag="bk")
            nc.sync.dma_start(out=bk, in_=qkb[128:256])
            if has_v_bias:
                vb_sb = consts.tile([128, 128], F32, tag="vbsb")
                nc.sync.dma_start(
                    out=vb_sb,
                    in_=bass.AP(tensor=vb, offset=0, ap=[[0, 128], [1, 128]]),
                )
            eps16 = consts.tile([16, 1], F32, tag="eps16")
            nc.vector.memset(eps16, EPS)
            eighth = consts.tile([16, 1], F32, tag="eighth")
            nc.vector.memset(eighth, 0.125)
            expb = consts.tile([128, 1], F32, tag="expb")
            nc.vector.memset(expb, EXPB)

            for _rep in range(chain):
                # ---- phase A: GroupNorm stats ----
                h_t = []
                scale_t, bias_t = [], []
                with (
                    tc.tile_pool(name="ps_g", bufs=2, space="PSUM") as ps_g,
                    tc.tile_pool(name="ps_warm", bufs=2, space="PSUM") as ps_warm,
                ):
                    # dummy Ln: preloads the natural_log act table while the
                    # x DMA is still in flight.
                    dummy = st.tile([16, 1], F32, tag="dummy")
                    nc.scalar.activation(dummy, eps16,
                                         mybir.ActivationFunctionType.Ln, scale=1.0)
                    x_t_list = []
                    gsb_ts, var_ts = [], []
                    for t in range(2):
                        x_t = xh_pool.tile([128, N], F32, tag="x", name=f"x{t}")
                        stats = st.tile([128, 8, 6], F32, tag="stats")
                        xr = x_t.rearrange("p (k f) -> p k f", f=512)
                        for q4 in range(4):
                            qs = slice(q4 * (N // 4), (q4 + 1) * (N // 4))
                            nc.sync.dma_start(out=x_t[:, qs],
                                              in_=xb[t * 128:(t + 1) * 128, qs])
                            # junk matmul on the freshly-landed chunk: keeps
                            # the PE HAM activity monitor warm through the
                            # DMA/stats prologue (output is never read).
                            wj = ps_warm.tile([16, 512], F32, tag="warm")
                            nc.tensor.matmul(wj, lhsT=g_sb,
                                             rhs=x_t[:, q4 * 1024:q4 * 1024 + 512],
                                             start=True, stop=True)
                            # stats per landed quarter - overlaps the DMA
                            for k in (2 * q4, 2 * q4 + 1):
                                nc.vector.bn_stats(out=stats[:, k, :],
                                                   in_=xr[:, k, :])
                        x_t_list.append(x_t)
                        mv = st.tile([128, 2], F32, tag="mv")
                        nc.vector.bn_aggr(out=mv, in_=stats)

                        # mq = [mean_c, var_c + mean_c^2]
                        mq = st.tile([128, 2], F32, tag="mq")
                        nc.vector.tensor_copy(mq[:, 0:1], mv[:, 0:1])
                        sq = st.tile([128, 1], F32, tag="sq")
                        nc.vector.tensor_mul(sq, mv[:, 0:1], mv[:, 0:1])
                        nc.vector.tensor_add(mq[:, 1:2], mv[:, 1:2], sq)

                        gps = ps_g.tile([16, 2], F32, tag="gps")
                        nc.tensor.matmul(gps, lhsT=g_sb, rhs=mq, start=True, stop=True)
                        # per-group E[x], E[x^2]
                        gsb = st.tile([16, 2], F32, tag=f"gsb2_{t}")
                        nc.vector.tensor_mul(gsb, gps, eighth.to_broadcast((16, 2)))
                        sqg = st.tile([16, 1], F32, tag="sqg")
                        nc.vector.tensor_mul(sqg, gsb[:, 0:1], gsb[:, 0:1])
                        var = st.tile([16, 1], F32, tag=f"var{t}")
                        nc.vector.tensor_sub(var, gsb[:, 1:2], sqg)
                        gsb_ts.append(gsb)
                        var_ts.append(var)

                    # rstd = exp(-0.5 * ln(var + eps)), grouped Ln,Ln / Exp,Exp
                    # so ScalarE loads each act table at most once.
                    lnv_ts = []
                    for t in range(2):
                        lnv = st.tile([16, 1], F32, tag=f"lnv{t}")
                        nc.scalar.activation(lnv, var_ts[t],
                                             mybir.ActivationFunctionType.Ln,
                                             bias=eps16, scale=1.0)
                        lnv_ts.append(lnv)
                    rstd_ts = []
                    for t in range(2):
                        rstd = st.tile([16, 1], F32, tag=f"rstd{t}")
                        nc.scalar.activation(rstd, lnv_ts[t],
                                             mybir.ActivationFunctionType.Exp,
                                             scale=-0.5)
                        rstd_ts.append(rstd)

                    for t in range(2):
                        ms = st.tile([16, 2], F32, tag=f"ms{t}")
                        nc.vector.tensor_copy(ms[:, 0:1], gsb_ts[t][:, 0:1])
                        nc.vector.tensor_copy(ms[:, 1:2], rstd_ts[t])

                        # broadcast per-group stats to per-channel [128,2] via
                        # a tiny PE matmul with the transposed group indicator.
                        bps = ps_g.tile([128, 2], F32, tag="bps")
                        nc.tensor.matmul(bps, lhsT=gt_sb, rhs=ms, start=True, stop=True)
                        sc = consts.tile([128, 1], F32, tag=f"scale{t}")
                        nc.vector.tensor_mul(sc, bps[:, 1:2], gw_t[t])
                        scale_t.append(sc)
                        tmp = st.tile([128, 1], F32, tag="tmpb")
                        nc.vector.tensor_mul(tmp, bps[:, 0:1], sc)
                        bi = consts.tile([128, 1], F32, tag=f"bias{t}")
                        nc.vector.tensor_sub(bi, gb_t[t], tmp)
                        bias_t.append(bi)

                # ---- phase B: GroupNorm h + QKV, interleaved per 512-column
                # chunk so the PE never waits multiple microseconds behind a
                # full-width DVE pass (which used to re-throttle the HAM).
                Q2 = qk_sb.tile([128, N], MADR, tag="q2")
                K2 = qk_sb.tile([128, N], MAD, tag="k2")
                # fp8 V^T, padded to 80 cols so the DoubleRow ko-stride (80 B)
                # is a multiple of 16
                V1 = v1p.tile([128, MB, 2, 80], F8, tag="v1")
                for t in range(2):
                    h_new = hp_pool.tile([128, N], MMD, tag="h", name=f"h{t}")
                    h_t.append(h_new)
                with tc.tile_pool(name="ps_qkv", bufs=3, space="PSUM") as ps_qkv:
                    for nb in range(NB):
                        ns = slice(nb * 512, (nb + 1) * 512)
                        for t in range(2):
                            # h = x * scale + bias (one 512-col chunk)
                            nc.vector.tensor_scalar(
                                out=h_t[t][:, ns], in0=x_t_list[t][:, ns],
                                scalar1=scale_t[t], scalar2=bias_t[t],
                                op0=mybir.AluOpType.mult, op1=mybir.AluOpType.add,
                            )
                        if nb == 0:
                            nc.vector.memset(V1, 1.0)
                        for dst, col0, bias_ap in ((Q2, 0, bq), (K2, 128, bk)):
                            ps = ps_qkv.tile([128, 512], F32, tag="mm")
                            nc.tensor.matmul(ps, lhsT=w_qk[0][:, col0:col0 + 128],
                                             rhs=h_t[0][:, ns], start=True, stop=False)
                            nc.tensor.matmul(ps, lhsT=w_qk[1][:, col0:col0 + 128],
                                             rhs=h_t[1][:, ns], start=False, stop=True)
                            # bias-add on the otherwise-idle ScalarE (Identity
                            # is in every act table set - no table reload).
                            nc.scalar.activation(
                                dst[:, ns], ps, mybir.ActivationFunctionType.Identity,
                                bias=bias_ap, scale=1.0)
                        # V^T chunks for this 512-col chunk (frees the proj
                        # PSUM banks for a 3-deep qk pipeline in phase C)
                        for q4 in range(4):
                            mbv = nb * 4 + q4
                            psv = ps_qkv.tile([128, 128], F32, tag="mm",
                                              name=f"psv_{mbv}")
                            cs = slice(mbv * 128, (mbv + 1) * 128)
                            nc.tensor.matmul(psv, lhsT=h_t[0][:, cs],
                                             rhs=w_v[0], start=True, stop=False)
                            nc.tensor.matmul(psv, lhsT=h_t[1][:, cs],
                                             rhs=w_v[1], start=False, stop=True)
                            if has_v_bias:
                                nc.vector.tensor_add(
                                    V1[:, mbv, :, 0:64],
                                    psv.rearrange("p (h d) -> p h d", h=2),
                                    vb_sb.rearrange("p (h d) -> p h d", h=2),
                                )
                            else:
                                nc.vector.tensor_copy(
                                    V1[:, mbv, :, 0:64],
                                    psv.rearrange("p (h d) -> p h d", h=2),
                                )

                # ---- phase C: attention ----
                A_lo = xh_pool.tile([64, N], MMD, tag="x", name="A_lo")
                A_hi = xh_pool.tile([64, N], MMD, tag="x", name="A_hi")
                with (
                    tc.tile_pool(name="ps_qk", bufs=3, space="PSUM") as ps_qk,
                    tc.tile_pool(name="ps_pv", bufs=2, space="PSUM") as ps_pv,
                ):
                    recs, bcs = {}, {}

                    def emit_normalize(nb2, h2):
                        # broadcast the raw denominator across all partitions
                        # by bouncing through DRAM (a DRAM AP may have a
                        # zero-step partition dim; SBUF cannot), then take the
                        # fast approx reciprocal on the [128, 512] broadcast.
                        # (reciprocal_approx_fast is base-partition-0 only: at
                        # other bases the custom-DVE op reads garbage on HW.)
                        pvs = recs[nb2][h2]
                        dr = drec.tile([1, 512], F32, tag="dr")
                        nc.sync.dma_start(out=dr, in_=pvs[64:65, :])
                        bcr = bcp.tile([128, 512], F32, tag="bcr")
                        nc.sync.dma_start(
                            out=bcr,
                            in_=bass.AP(tensor=dr.tensor, offset=dr.offset,
                                        ap=[[0, 128]] + list(dr.ap[1:])),
                        )
                        bc = bcp.tile([128, 512], F32, tag="bc")
                        nc.vector.reciprocal_approx_fast(out=bc, in_=bcr)
                        bcs[(nb2, h2)] = (bc, pvs)

                    def emit_scale(nb, h2):
                        # A = raw_pv * 1/den ([64, 512] per head - half the
                        # elements of scaling the proj outputs); off the PE
                        # critical path, the bc broadcast had a head start.
                        ns = slice(nb * 512, (nb + 1) * 512)
                        bc, pvs = bcs.pop((nb, h2))
                        A = A_lo if h2 == 0 else A_hi
                        nc.vector.tensor_mul(A[:, ns], pvs[0:64, :], bc[0:64, :])

                    def emit_qk(nb, mb):
                        ns = slice(nb * 512, (nb + 1) * 512)
                        ms_ = slice(mb * 128, (mb + 1) * 128)
                        qk = ps_qk.tile([128, 1024], F32, tag="qk", name=f"qk_{nb}_{mb}")
                        nc.tensor.matmul(qk[:, 0:512], lhsT=K2[0:64, ms_],
                                         rhs=Q2[0:64, ns], start=True, stop=True,
                                         skip_group_check=True)
                        nc.tensor.matmul(qk[:, 512:1024], lhsT=K2[64:128, ms_],
                                         rhs=Q2[64:128, ns], start=True, stop=True,
                                         skip_group_check=True)
                        return qk

                    # software pipeline: emit iteration i+1's QK matmuls before
                    # iteration i's PV matmuls, so the in-order PE queue never
                    # stalls behind a PV that waits on ScalarE's exp.
                    iters = [(nb, mb) for nb in range(NB) for mb in range(MB)]
                    pv_tiles = {}
                    pending_pv = None
                    qk_fifo = [emit_qk(*iters[0]), emit_qk(*iters[1]),
                               emit_qk(*iters[2])]
                    for idx, (nb, mb) in enumerate(iters):
                        ns = slice(nb * 512, (nb + 1) * 512)
                        if nb > 0 and mb in (6, 10, 14, 18):
                            # spread the per-head normalize/scale ops out so
                            # they never bunch up in the DVE queue
                            (emit_normalize if mb < 14 else emit_scale)(
                                nb - 1, 0 if mb in (6, 14) else 1)
                        if mb == 0:
                            pv_lo = ps_pv.tile([65, 512], F32, tag="pv", name=f"pvlo_{nb}")
                            pv_hi = ps_pv.tile([65, 512], F32, tag="pv", name=f"pvhi_{nb}")
                            pv_tiles[nb] = (pv_lo, pv_hi)
                        pv_lo, pv_hi = pv_tiles[nb]
                        qk_cur = qk_fifo.pop(0)
                        ko = mb % 2
                        # chunk-granular engine split: whole [128,1024] exp of
                        # an even chunk on ScalarE (exact, fp8e4 out), of an
                        # odd chunk on the DVE (Schraudolph int8 bits).  One
                        # op per chunk halves the per-op overhead, and each
                        # QK gates on a single engine's completion.  (Failed
                        # at qk-depth 2; retested at depth 3 where exps run
                        # ahead of consumption.)
                        if ko == 0:
                            pexp_cur = pexpa.tile([128, 2, 1024], F8, tag="pexpa",
                                                  name=f"pexp_{nb}_{mb}")
                            nc.scalar.activation(pexp_cur[:, 0, :], qk_cur,
                                                 mybir.ActivationFunctionType.Exp,
                                                 scale=EXPSC, bias=expb)
                        else:
                            nc.vector.tensor_scalar(
                                out=pexp_cur.bitcast(I8)[:, 1, :], in0=qk_cur,
                                scalar1=-SCHROFF, scalar2=SCHROFF,
                                op0=mybir.AluOpType.max, op1=mybir.AluOpType.add,
                            )
                        # 3-deep QK pipeline: QK_{i+3} reuses qk_i's PSUM
                        # banks, so two QKs can run during one exp and the
                        # engines' exps go back-to-back.
                        if idx + 3 < len(iters):
                            qk_fifo.append(emit_qk(*iters[idx + 3]))
                        def emit_pv(p_nb, p_pair, p_pexp, h):
                            # DoubleRow PV: one matmul per head contracts the
                            # chunk PAIR (2 x 128 keys) at once.
                            pv = pv_tiles[p_nb][h]
                            nc.tensor.matmul(
                                pv,
                                lhsT=V1[:, 2 * p_pair:2 * p_pair + 2, h, 0:65],
                                rhs=p_pexp[:, :, h * 512:(h + 1) * 512],
                                start=(p_pair == 0),
                                stop=(p_pair == MB // 2 - 1),
                                perf_mode=mybir.MatmulPerfMode.DoubleRow,
                                skip_group_check=True)
                            if p_pair == MB // 2 - 1:
                                # Release this pv PSUM bank with a single
                                # [65,512] ScalarE copy: rows 0:64 raw A,
                                # row 64 the denominator (same per-lane cost
                                # as copying A alone; no DVE involvement).
                                pv2 = pv_tiles[p_nb][h]
                                pvs = denp.tile([65, 512], F32, tag="den")
                                nc.scalar.activation(
                                    pvs, pv2,
                                    mybir.ActivationFunctionType.Identity,
                                    scale=1.0)
                                recs.setdefault(p_nb, []).append(pvs)

                        # both PVs run one iteration after the pair's exps
                        # were issued (identical gates in chunk-split mode),
                        # so the in-order PE queue never waits on them.
                        if pending_pv is not None:
                            emit_pv(*pending_pv, 0)
                            emit_pv(*pending_pv, 1)
                            pending_pv = None
                        if ko == 1:
                            pending_pv = (nb, mb // 2, pexp_cur)

                    # flush the final PV pair + the last block's
                    # normalize/scale (inside the pool scope)
                    emit_pv(*pending_pv, 0)
                    emit_pv(*pending_pv, 1)
                    for h2 in range(2):
                        emit_normalize(NB - 1, h2)
                        emit_scale(NB - 1, h2)

                # ---- proj tail: runs after the attention pools close, in
                # the PSUM banks they freed ----
                with tc.tile_pool(name="ps_pj", bufs=2, space="PSUM") as ps_pj:
                    for nbp in range(NB):
                        ns = slice(nbp * 512, (nbp + 1) * 512)
                        for m in range(2):
                            ps_y = ps_pj.tile([128, 512], F32, tag="pj")
                            nc.tensor.matmul(ps_y, lhsT=wp_lo[:, m * 128:(m + 1) * 128],
                                             rhs=A_lo[:, ns], start=True, stop=False)
                            nc.tensor.matmul(ps_y, lhsT=wp_hi[:, m * 128:(m + 1) * 128],
                                             rhs=A_hi[:, ns], start=False, stop=True)
                            y_sb = yout.tile([128, 512], F32, tag="y")
                            if m == 0:
                                nc.scalar.activation(
                                    y_sb, ps_y,
                                    mybir.ActivationFunctionType.Identity,
                                    scale=1.0)
                            else:
                                nc.vector.tensor_copy(y_sb, ps_y)
                            nc.sync.dma_start(out=yp[m * 128:(m + 1) * 128, ns],
                                              in_=y_sb)

    nc.finalize()
    return nc


_CACHE = {}


ATTN_DTYPE = BF16
MM_DTYPE = BF16


def _get_program(has_v_bias: bool, chain: int = 1):
    key = ("prog", has_v_bias, str(ATTN_DTYPE), str(MM_DTYPE), chain)
    if key not in _CACHE:
        _CACHE[key] = _build_program(has_v_bias, ATTN_DTYPE, MM_DTYPE, chain)
    return _CACHE[key]


def _make_in_maps(x, gn_w, gn_b, qkv_w, qkv_b, proj_w):
    x = np.ascontiguousarray(x, dtype=np.float32)
    in_maps = []
    for core in range(NCORES):
        b, p = core // 2, core % 2
        rows_q = slice(p * 128, (p + 1) * 128)
        rows_k = slice(256 + p * 128, 256 + (p + 1) * 128)
        rows_v = slice(512 + p * 128, 512 + (p + 1) * 128)
        m = {
            "xb": np.ascontiguousarray(x[b].reshape(C, N)),
            "wqkT": np.ascontiguousarray(
                np.concatenate([qkv_w[rows_q] * LOG2E, qkv_w[rows_k]],
                               axis=0).T.astype(np.float32)),
            "wvT": np.ascontiguousarray(qkv_w[rows_v].T.astype(np.float32)),
            "wpT": np.ascontiguousarray(proj_w[:, p * 128:(p + 1) * 128].T.astype(np.float32)),
            "gnw": np.ascontiguousarray(gn_w.astype(np.float32)),
            "gnb": np.ascontiguousarray(gn_b.astype(np.float32)),
            "qkb": np.ascontiguousarray(
                np.concatenate([qkv_b[rows_q] * LOG2E,
                                qkv_b[rows_k]]).astype(np.float32)),
        }
        if np.any(qkv_b[512:768]):
            m["vb"] = np.ascontiguousarray(qkv_b[rows_v].astype(np.float32))
        in_maps.append(m)
    return in_maps


def _get_executor(nc, donate=True):
    """Build (once) a cached jitted 8-core executor for the program.

    Mirrors concourse.bass2jax.run_bass_via_pjrt, but caches the jitted
    callable so repeat kernel() calls don't re-trace/re-compile the XLA
    wrapper.  Returns (fn, in_names, out_names) where fn takes a list of
    per-core input dicts and returns a list of per-core output dicts.
    """
    key = ("exec", id(nc), donate)
    if key in _CACHE:
        return _CACHE[key]
    import jax
    import concourse.mybir as _mybir
    from jax.experimental.shard_map import shard_map
    from jax.sharding import Mesh, PartitionSpec
    from concourse import bass2jax

    bass2jax.install_neuronx_cc_hook()
    partition_name = nc.partition_id_tensor.name if nc.partition_id_tensor else None
    in_names, out_names, out_avals, zero_outs = [], [], [], []
    for alloc in nc.m.functions[0].allocations:
        if not isinstance(alloc, _mybir.MemoryLocationSet):
            continue
        name = alloc.memorylocations[0].name
        if alloc.kind == "ExternalInput":
            if name != partition_name:
                in_names.append(name)
        elif alloc.kind == "ExternalOutput":
            shape = tuple(alloc.tensor_shape)
            dtype = _mybir.dt.np(alloc.dtype)
            out_names.append(name)
            out_avals.append(jax.core.ShapedArray(shape, dtype))
            zero_outs.append(np.zeros(shape, dtype))
    n_params = len(in_names)
    n_outs = len(out_avals)
    all_names = in_names + out_names + ([partition_name] if partition_name else [])

    def _body(*args):
        operands = list(args)
        if partition_name is not None:
            operands.append(bass2jax.partition_id_tensor())
        return tuple(bass2jax._bass_exec_p.bind(
            *operands,
            out_avals=tuple(out_avals),
            in_names=tuple(all_names),
            out_names=tuple(out_names),
            lowering_input_output_aliases=(),
            sim_require_finite=True,
            sim_require_nnan=True,
            nc=nc,
        ))

    devices = jax.devices()[:NCORES]
    mesh = Mesh(np.asarray(devices), ("core",))
    in_specs = (PartitionSpec("core"),) * (n_params + n_outs)
    out_specs = (PartitionSpec("core"),) * n_outs
    donate_idx = tuple(range(n_params, n_params + n_outs)) if donate else ()
    sharded = jax.jit(
        shard_map(_body, mesh=mesh, in_specs=in_specs, out_specs=out_specs,
                  check_rep=False),
        donate_argnums=donate_idx, keep_unused=True,
    )

    _CACHE[("sharded", id(nc))] = sharded
    _CACHE[("zeros", id(nc))] = [((NCORES * z.shape[0],) + z.shape[1:], z.dtype)
                                 for z in zero_outs]

    def fn(in_maps):
        concat_in = [
            np.concatenate([np.asarray(in_maps[c][nm]) for c in range(NCORES)], axis=0)
            for nm in in_names
        ]
        concat_zeros = [
            np.zeros((NCORES * z.shape[0], *z.shape[1:]), z.dtype) for z in zero_outs
        ]
        out_arrs = sharded(*concat_in, *concat_zeros)
        return [
            {nm: np.asarray(out_arrs[i]).reshape(NCORES, *out_avals[i].shape)[c]
             for i, nm in enumerate(out_names)}
            for c in range(NCORES)
        ]

    _CACHE[key] = (fn, in_names, out_names)
    return _CACHE[key]


def _prep(inputs):
    x = np.asarray(inputs["x"], dtype=np.float32)
    qkv_b = np.asarray(inputs["qkv_b"], dtype=np.float32)
    has_v_bias = bool(np.any(qkv_b[512:768]))
    nc = _get_program(has_v_bias)
    in_maps = _make_in_maps(
        x,
        np.asarray(inputs["gn_w"], dtype=np.float32),
        np.asarray(inputs["gn_b"], dtype=np.float32),
        np.asarray(inputs["qkv_w"], dtype=np.float32),
        qkv_b,
        np.asarray(inputs["proj_w"], dtype=np.float32),
    )
    return nc, in_maps, x


def run(inputs, trace=False):
    """Run the sharded kernel.  Returns (output, per-core results list)."""
    nc, in_maps, x = _prep(inputs)
    fn, _, _ = _get_executor(nc)
    results = fn(in_maps)
    proj_b = np.asarray(inputs["proj_b"], dtype=np.float32)
    parts = [results[c]["yp"] for c in range(NCORES)]
    y = np.stack([parts[2 * b] + parts[2 * b + 1] for b in range(B)])  # [B, C, N]
    y = y + proj_b[None, :, None]
    out = np.asarray(inputs["x"], dtype=np.float32) + y.reshape(B, C, 64, 64)
    return out.astype(np.float32), results


def kernel(**inputs) -> np.ndarray:
    out, _ = run(inputs, trace=False)
    return out



# revision 70
# speedup vs baseline: 1.0131x; 1.0131x over previous
"""Trainium2 Bass kernel for nn_AttentionBlock (GroupNorm + 4-head attention + proj + residual).

Sharding: 8 cores = (batch b in 0..3) x (head-pair p in 0..1).  Each core gets
x[b] and the weight slices for its two heads, computes GroupNorm -> QKV ->
attention -> partial proj (over its 128 attention-output channels), and returns
a partial [256, 4096] projection output.  The host sums the two partials per
batch, adds proj bias + residual, and reshapes.

The device program is identical on all cores (SPMD); all per-core variation is
carried by the input tensors.

Layout notes (per core):
  h   = groupnorm(x[b])                SBUF [c=128 x2, n=4096] bf16
  Q2  = log2e * (Wq_pair^T . h + bq)   SBUF [128 (2 heads x 64 d), 4096] bf16
  K2  = Wk_pair^T . h + bk             SBUF [128, 4096] bf16
  V1  = [h^T . Wv_pair | ones]         SBUF [128 (m-chunk), 32, 2, 80] fp8e4
  scores'^T[m, n] = sum_d K[d,m] Q[d,n] via matmul(lhsT=K2[64 rows], rhs=Q2),
     two heads packed concurrently in PE row-groups (0,0) and (64,0);
     = log2e * raw scores thanks to the host-side Q pre-scale.
  pexp (fp8e4, ~exp(0.125*raw)*2^(-C/8)) is produced half-and-half:
     ScalarE: exact Exp with free affine (scale 0.125/log2e, bias -C/8*ln2)
     DVE:     Schraudolph bits8 = clamp(qk' + 56 - C) via one tensor_scalar
              (max, add) written through an int8 view of the fp8 tile.
     NB fp8e4 here is IEEE-style: exponent 15 = Inf/NaN, so C keeps bits <= ~116.
  pv[d+1, n] += V1^T . pexp            fp8 DoubleRow matmuls, each contracting
     a PAIR of 128-key chunks (lhsT [128,2,65], rhs [128,2,512]); row 64
     (the ones column) accumulates the softmax denominator.
  At the end of each n-block: den rows copied out on the DVE, raw A on
  ScalarE (frees the PSUM banks fast), then off the critical path:
  DRAM-bounce broadcast of den, approx-reciprocal on the [128,512] broadcast
  (base-partition 0 only - the custom-DVE op breaks at other bases), A *= 1/den,
  proj accumulates both heads into one PSUM tile per m.
  y_partial = Wp[:, pair]^T . A        [256, 4096] -> DRAM
"""

import numpy as np

import concourse.bacc as bacc
import concourse.bass as bass

import concourse.mybir as mybir
import concourse.tile as tile

B = 4
C = 256
N = 4096          # 64*64
NH = 4
D = 64            # head dim
GROUPS = 32
EPS = 1e-5
NCORES = 8
SCALE = float(D) ** -0.5  # 0.125
F32 = mybir.dt.float32
F32R = mybir.dt.float32r
BF16 = mybir.dt.bfloat16
F8 = mybir.dt.float8e4
I8 = mybir.dt.int8

MB = 32           # m chunks of 128
NB = 8            # n chunks of 512

# fp8-e4m3 softmax weights + DoubleRow PV.
#
# The Q weights/bias are pre-scaled by log2(e) on the host, so the QK matmul
# emits qk' = log2e * qk_raw, i.e. 8*t where exp(0.125*qk_raw) = 2^t.
#  - ScalarE half (head-lo): exact exp -> fp8e4 out, scaled by 2^(-C/8) via
#    the activation's free affine (scale = 0.125/log2e, bias = -C/8*ln2).
#  - DVE half (head-hi): Schraudolph bits8 = clamp(qk' + 56 - C, >=0) via one
#    tensor_scalar (op0 = max, op1 = add) written through an int8 view.
# The 2^(-C/8) factor cancels in the softmax normalization.  PV runs fp8
# DoubleRow: each matmul contracts TWO 128-key chunks at once (rhs [128,2,512],
# lhsT [128,2,65]), halving the PV instruction count.
LOG2E = 1.4426950408889634
LN2 = 0.6931471805599453
F8C = 24.0                      # exponent offset: bits8 <= ~116 (qk' max ~84); fp8e4 here is IEEE-style, exponent 15 (bits >= 120) = Inf/NaN
EXPSC = 0.125 / LOG2E           # ACT exp input scale (qk' -> 0.125*qk_raw)
EXPB = -(F8C / 8.0) * LN2       # ACT exp input bias  (output *= 2^(-C/8))
SCHROFF = 56.0 - F8C            # DVE: bits8 = max(qk', -SCHROFF) + SCHROFF


def _build_program(has_v_bias: bool, attn_dtype=BF16, mm_dtype=BF16, chain: int = 1):
    nc = bacc.Bacc("TRN2", target_bir_lowering=False)
    MMD = mm_dtype        # dtype of tiles feeding qkv/proj matmuls (h, weights, A)
    MAD = attn_dtype      # dtype of lhsT tiles for QK/PV (K2, V1)
    MADR = attn_dtype     # dtype of rhs tiles for QK/PV (Q2, pexp)

    xb = nc.dram_tensor("xb", [C, N], F32, kind="ExternalInput")
    wqkT = nc.dram_tensor("wqkT", [C, 256], F32, kind="ExternalInput")   # cols 0:128 Qpair, 128:256 Kpair
    wvT = nc.dram_tensor("wvT", [C, 128], F32, kind="ExternalInput")
    wpT = nc.dram_tensor("wpT", [128, C], F32, kind="ExternalInput")     # rows = pair channels
    gnw = nc.dram_tensor("gnw", [C], F32, kind="ExternalInput")
    gnb = nc.dram_tensor("gnb", [C], F32, kind="ExternalInput")
    qkb = nc.dram_tensor("qkb", [256], F32, kind="ExternalInput")        # 0:128 Q bias pair, 128:256 K bias pair
    if has_v_bias:
        vb = nc.dram_tensor("vb", [128], F32, kind="ExternalInput")
    yp = nc.dram_tensor("yp", [C, N], F32, kind="ExternalOutput")

    # group-indicator matrix: G[c, g] = 1 if c // 8 == g else 0  (per c-tile)
    g_host = np.zeros((128, 16), dtype=np.float32)
    for c in range(128):
        g_host[c, c // 8] = 1.0
    g_dram = nc.inline_tensor(g_host, name="gmat")
    gt_dram = nc.inline_tensor(np.ascontiguousarray(g_host.T), name="gmatT")

    with tile.TileContext(nc) as tc:
        with (
            tc.tile_pool(name="consts", bufs=1) as consts,
            tc.tile_pool(name="xh", bufs=2) as xh_pool,
            tc.tile_pool(name="hp", bufs=2) as hp_pool,
            tc.tile_pool(name="qk_sb", bufs=1) as qk_sb,
            tc.tile_pool(name="v1p", bufs=1) as v1p,
            tc.tile_pool(name="st", bufs=2) as st,
            tc.tile_pool(name="pexpa", bufs=6) as pexpa,
            tc.tile_pool(name="denp", bufs=4) as denp,
            tc.tile_pool(name="bcp", bufs=8) as bcp,
            tc.tile_pool(name="drec", bufs=4, space="DRAM") as drec,
            tc.tile_pool(name="yout", bufs=4) as yout,
        ):
            # ---- constants ----
            g_sb = consts.tile([128, 16], F32, tag="gsb")
            nc.sync.dma_start(out=g_sb, in_=g_dram[:, :])
            gt_sb = consts.tile([16, 128], F32, tag="gtsb")
            nc.sync.dma_start(out=gt_sb, in_=gt_dram[:, :])

            def load_weight(shape, dram_slice, tag):
                w_raw = consts.tile(shape, F32, tag=tag + "_r", name=tag + "_r")
                nc.sync.dma_start(out=w_raw, in_=dram_slice)
                if MMD == F32:
                    return w_raw
                w_c = consts.tile(shape, MMD, tag=tag, name=tag)
                nc.vector.tensor_copy(w_c, w_raw)
                return w_c

            w_qk = [load_weight([128, 256], wqkT[t * 128:(t + 1) * 128, :], f"wqk{t}")
                    for t in range(2)]
            w_v = [load_weight([128, 128], wvT[t * 128:(t + 1) * 128, :], f"wv{t}")
                   for t in range(2)]
            wp_lo = load_weight([64, 256], wpT[0:64, :], "wplo")
            wp_hi = load_weight([64, 256], wpT[64:128, :], "wphi")

            gw_t, gb_t = [], []
            for t in range(2):
                gwt = consts.tile([128, 1], F32, tag=f"gw{t}")
                nc.sync.dma_start(out=gwt, in_=gnw[t * 128:(t + 1) * 128])
                gw_t.append(gwt)
                gbt = consts.tile([128, 1], F32, tag=f"gb{t}")
                nc.sync.dma_start(out=gbt, in_=gnb[t * 128:(t + 1) * 128])
                gb_t.append(gbt)
            bq = consts.tile([128, 1], F32, tag="bq")
            nc.sync.dma_start(out=bq, in_=qkb[0:128])
            bk = consts.tile([128, 1], F32, tag="bk")
            nc.sync.dma_start(out=bk, in_=qkb[128:256])
            if has_v_bias:
                vb_sb = consts.tile([128, 128], F32, tag="vbsb")
                nc.sync.dma_start(
                    out=vb_sb,
                    in_=bass.AP(tensor=vb, offset=0, ap=[[0, 128], [1, 128]]),
                )
            eps16 = consts.tile([16, 1], F32, tag="eps16")
            nc.vector.memset(eps16, EPS)
            eighth = consts.tile([16, 1], F32, tag="eighth")
            nc.vector.memset(eighth, 0.125)
            expb = consts.tile([128, 1], F32, tag="expb")
            nc.vector.memset(expb, EXPB)

            for _rep in range(chain):
                # ---- phase A: GroupNorm stats ----
                h_t = []
                scale_t, bias_t = [], []
                with (
                    tc.tile_pool(name="ps_g", bufs=2, space="PSUM") as ps_g,
                    tc.tile_pool(name="ps_warm", bufs=2, space="PSUM") as ps_warm,
                ):
                    # dummy Ln: preloads the natural_log act table while the
                    # x DMA is still in flight.
                    dummy = st.tile([16, 1], F32, tag="dummy")
                    nc.scalar.activation(dummy, eps16,
                                         mybir.ActivationFunctionType.Ln, scale=1.0)
                    x_t_list = []
                    gsb_ts = []
                    var2 = st.tile([16, 2], F32, tag="var2")
                    for t in range(2):
                        x_t = xh_pool.tile([128, N], F32, tag="x", name=f"x{t}")
                        stats = st.tile([128, 8, 6], F32, tag="stats")
                        xr = x_t.rearrange("p (k f) -> p k f", f=512)
                        for q4 in range(4):
                            qs = slice(q4 * (N // 4), (q4 + 1) * (N // 4))
                            nc.sync.dma_start(out=x_t[:, qs],
                                              in_=xb[t * 128:(t + 1) * 128, qs])
                            # junk matmul on the freshly-landed chunk: keeps
                            # the PE HAM activity monitor warm through the
                            # DMA/stats prologue (output is never read).
                            wj = ps_warm.tile([16, 512], F32, tag="warm")
                            nc.tensor.matmul(wj, lhsT=g_sb,
                                             rhs=x_t[:, q4 * 1024:q4 * 1024 + 512],
                                             start=True, stop=True)
                            # stats per landed quarter - overlaps the DMA
                            for k in (2 * q4, 2 * q4 + 1):
                                nc.vector.bn_stats(out=stats[:, k, :],
                                                   in_=xr[:, k, :])
                        x_t_list.append(x_t)
                        mv = st.tile([128, 2], F32, tag="mv")
                        nc.vector.bn_aggr(out=mv, in_=stats)

                        # mq = [mean_c, var_c + mean_c^2]
                        mq = st.tile([128, 2], F32, tag="mq")
                        nc.vector.tensor_copy(mq[:, 0:1], mv[:, 0:1])
                        sq = st.tile([128, 1], F32, tag="sq")
                        nc.vector.tensor_mul(sq, mv[:, 0:1], mv[:, 0:1])
                        nc.vector.tensor_add(mq[:, 1:2], mv[:, 1:2], sq)

                        gps = ps_g.tile([16, 2], F32, tag="gps")
                        nc.tensor.matmul(gps, lhsT=g_sb, rhs=mq, start=True, stop=True)
                        # per-group E[x], E[x^2]
                        gsb = st.tile([16, 2], F32, tag=f"gsb2_{t}")
                        nc.vector.tensor_mul(gsb, gps, eighth.to_broadcast((16, 2)))
                        sqg = st.tile([16, 1], F32, tag="sqg")
                        nc.vector.tensor_mul(sqg, gsb[:, 0:1], gsb[:, 0:1])
                        nc.vector.tensor_sub(var2[:, t:t + 1], gsb[:, 1:2], sqg)
                        gsb_ts.append(gsb)

                    # rstd = exp(-0.5 * ln(var + eps)).  Ln and Exp live in
                    # DIFFERENT act-table sets on this stack (each switch is a
                    # ~2.8 us reload), and the Tile scheduler reorders split
                    # per-t calls into Ln,Exp,Ln,Exp (3 reloads).  Fusing both
                    # groups into single [16,2] ops pins it to one reload.
                    lnv2 = st.tile([16, 2], F32, tag="lnv2")
                    nc.scalar.activation(lnv2, var2,
                                         mybir.ActivationFunctionType.Ln,
                                         bias=eps16, scale=1.0)
                    rstd2 = st.tile([16, 2], F32, tag="rstd2")
                    nc.scalar.activation(rstd2, lnv2,
                                         mybir.ActivationFunctionType.Exp,
                                         scale=-0.5)

                    for t in range(2):
                        ms = st.tile([16, 2], F32, tag=f"ms{t}")
                        nc.vector.tensor_copy(ms[:, 0:1], gsb_ts[t][:, 0:1])
                        nc.vector.tensor_copy(ms[:, 1:2], rstd2[:, t:t + 1])

                        # broadcast per-group stats to per-channel [128,2] via
                        # a tiny PE matmul with the transposed group indicator.
                        bps = ps_g.tile([128, 2], F32, tag="bps")
                        nc.tensor.matmul(bps, lhsT=gt_sb, rhs=ms, start=True, stop=True)
                        sc = consts.tile([128, 1], F32, tag=f"scale{t}")
                        nc.vector.tensor_mul(sc, bps[:, 1:2], gw_t[t])
                        scale_t.append(sc)
                        tmp = st.tile([128, 1], F32, tag="tmpb")
                        nc.vector.tensor_mul(tmp, bps[:, 0:1], sc)
                        bi = consts.tile([128, 1], F32, tag=f"bias{t}")
                        nc.vector.tensor_sub(bi, gb_t[t], tmp)
                        bias_t.append(bi)

                # ---- phase B: GroupNorm h + QKV, interleaved per 512-column
                # chunk so the PE never waits multiple microseconds behind a
                # full-width DVE pass (which used to re-throttle the HAM).
                Q2 = qk_sb.tile([128, N], MADR, tag="q2")
                K2 = qk_sb.tile([128, N], MAD, tag="k2")
                # fp8 V^T, padded to 80 cols so the DoubleRow ko-stride (80 B)
                # is a multiple of 16
                V1 = v1p.tile([128, MB, 2, 80], F8, tag="v1")
                for t in range(2):
                    h_new = hp_pool.tile([128, N], MMD, tag="h", name=f"h{t}")
                    h_t.append(h_new)
                with tc.tile_pool(name="ps_qkv", bufs=3, space="PSUM") as ps_qkv:
                    for nb in range(NB):
                        ns = slice(nb * 512, (nb + 1) * 512)
                        for t in range(2):
                            # h = x * scale + bias (one 512-col chunk)
                            nc.vector.tensor_scalar(
                                out=h_t[t][:, ns], in0=x_t_list[t][:, ns],
                                scalar1=scale_t[t], scalar2=bias_t[t],
                                op0=mybir.AluOpType.mult, op1=mybir.AluOpType.add,
                            )
                        if nb == 0:
                            nc.vector.memset(V1, 1.0)
                        for dst, col0, bias_ap in ((Q2, 0, bq), (K2, 128, bk)):
                            ps = ps_qkv.tile([128, 512], F32, tag="mm")
                            nc.tensor.matmul(ps, lhsT=w_qk[0][:, col0:col0 + 128],
                                             rhs=h_t[0][:, ns], start=True, stop=False)
                            nc.tensor.matmul(ps, lhsT=w_qk[1][:, col0:col0 + 128],
                                             rhs=h_t[1][:, ns], start=False, stop=True)
                            # bias-add on the otherwise-idle ScalarE (Identity
                            # is in every act table set - no table reload).
                            nc.scalar.activation(
                                dst[:, ns], ps, mybir.ActivationFunctionType.Identity,
                                bias=bias_ap, scale=1.0)
                        # V^T chunks for this 512-col chunk (frees the proj
                        # PSUM banks for a 3-deep qk pipeline in phase C)
                        for q4 in range(4):
                            mbv = nb * 4 + q4
                            psv = ps_qkv.tile([128, 128], F32, tag="mm",
                                              name=f"psv_{mbv}")
                            cs = slice(mbv * 128, (mbv + 1) * 128)
                            nc.tensor.matmul(psv, lhsT=h_t[0][:, cs],
                                             rhs=w_v[0], start=True, stop=False)
                            nc.tensor.matmul(psv, lhsT=h_t[1][:, cs],
                                             rhs=w_v[1], start=False, stop=True)
                            if has_v_bias:
                                nc.vector.tensor_add(
                                    V1[:, mbv, :, 0:64],
                                    psv.rearrange("p (h d) -> p h d", h=2),
                                    vb_sb.rearrange("p (h d) -> p h d", h=2),
                                )
                            else:
                                nc.vector.tensor_copy(
                                    V1[:, mbv, :, 0:64],
                                    psv.rearrange("p (h d) -> p h d", h=2),
                                )

                # ---- phase C: attention ----
                A_lo = xh_pool.tile([64, N], MMD, tag="x", name="A_lo")
                A_hi = xh_pool.tile([64, N], MMD, tag="x", name="A_hi")
                with (
                    tc.tile_pool(name="ps_qk", bufs=3, space="PSUM") as ps_qk,
                    tc.tile_pool(name="ps_pv", bufs=2, space="PSUM") as ps_pv,
                ):
                    recs, bcs = {}, {}

                    def emit_normalize(nb2, h2):
                        # broadcast the raw denominator across all partitions
                        # by bouncing through DRAM (a DRAM AP may have a
                        # zero-step partition dim; SBUF cannot), then take the
                        # fast approx reciprocal on the [128, 512] broadcast.
                        # (reciprocal_approx_fast is base-partition-0 only: at
                        # other bases the custom-DVE op reads garbage on HW.)
                        pvs = recs[nb2][h2]
                        dr = drec.tile([1, 512], F32, tag="dr")
                        nc.sync.dma_start(out=dr, in_=pvs[64:65, :])
                        bcr = bcp.tile([128, 512], F32, tag="bcr")
                        nc.sync.dma_start(
                            out=bcr,
                            in_=bass.AP(tensor=dr.tensor, offset=dr.offset,
                                        ap=[[0, 128]] + list(dr.ap[1:])),
                        )
                        bc = bcp.tile([128, 512], F32, tag="bc")
                        nc.vector.reciprocal_approx_fast(out=bc, in_=bcr)
                        bcs[(nb2, h2)] = (bc, pvs)

                    def emit_scale(nb, h2):
                        # A = raw_pv * 1/den ([64, 512] per head - half the
                        # elements of scaling the proj outputs); off the PE
                        # critical path, the bc broadcast had a head start.
                        ns = slice(nb * 512, (nb + 1) * 512)
                        bc, pvs = bcs.pop((nb, h2))
                        A = A_lo if h2 == 0 else A_hi
                        nc.vector.tensor_mul(A[:, ns], pvs[0:64, :], bc[0:64, :])

                    def emit_qk(nb, mb):
                        ns = slice(nb * 512, (nb + 1) * 512)
                        ms_ = slice(mb * 128, (mb + 1) * 128)
                        qk = ps_qk.tile([128, 1024], F32, tag="qk", name=f"qk_{nb}_{mb}")
                        nc.tensor.matmul(qk[:, 0:512], lhsT=K2[0:64, ms_],
                                         rhs=Q2[0:64, ns], start=True, stop=True,
                                         skip_group_check=True)
                        nc.tensor.matmul(qk[:, 512:1024], lhsT=K2[64:128, ms_],
                                         rhs=Q2[64:128, ns], start=True, stop=True,
                                         skip_group_check=True)
                        return qk

                    # software pipeline: emit iteration i+1's QK matmuls before
                    # iteration i's PV matmuls, so the in-order PE queue never
                    # stalls behind a PV that waits on ScalarE's exp.
                    iters = [(nb, mb) for nb in range(NB) for mb in range(MB)]
                    pv_tiles = {}
                    pending_pv = None
                    qk_fifo = [emit_qk(*iters[0]), emit_qk(*iters[1]),
                               emit_qk(*iters[2])]
                    for idx, (nb, mb) in enumerate(iters):
                        ns = slice(nb * 512, (nb + 1) * 512)
                        if nb > 0 and mb in (6, 10, 14, 18):
                            # spread the per-head normalize/scale ops out so
                            # they never bunch up in the DVE queue
                            (emit_normalize if mb < 14 else emit_scale)(
                                nb - 1, 0 if mb in (6, 14) else 1)
                        if mb == 0:
                            pv_lo = ps_pv.tile([65, 512], F32, tag="pv", name=f"pvlo_{nb}")
                            pv_hi = ps_pv.tile([65, 512], F32, tag="pv", name=f"pvhi_{nb}")
                            pv_tiles[nb] = (pv_lo, pv_hi)
                        pv_lo, pv_hi = pv_tiles[nb]
                        qk_cur = qk_fifo.pop(0)
                        ko = mb % 2
                        if ko == 0:
                            pexp_cur = pexpa.tile([128, 2, 1024], F8, tag="pexpa",
                                                  name=f"pexp_{nb}_{mb}")
                        # exp split across both engines: exact exp of the lo
                        # half on ScalarE (fp8e4 out), Schraudolph bits of the
                        # hi half on the DVE through an int8 view.
                        nc.scalar.activation(pexp_cur[:, ko, 0:512],
                                             qk_cur[:, 0:512],
                                             mybir.ActivationFunctionType.Exp,
                                             scale=EXPSC, bias=expb)
                        nc.vector.tensor_scalar(
                            out=pexp_cur.bitcast(I8)[:, ko, 512:1024],
                            in0=qk_cur[:, 512:1024],
                            scalar1=-SCHROFF, scalar2=SCHROFF,
                            op0=mybir.AluOpType.max, op1=mybir.AluOpType.add,
                        )
                        # 3-deep QK pipeline: QK_{i+3} reuses qk_i's PSUM
                        # banks, so two QKs can run during one exp and the
                        # engines' exps go back-to-back.
                        if idx + 3 < len(iters):
                            qk_fifo.append(emit_qk(*iters[idx + 3]))
                        def emit_pv(p_nb, p_pair, p_pexp, h):
                            # DoubleRow PV: one matmul per head contracts the
                            # chunk PAIR (2 x 128 keys) at once.
                            pv = pv_tiles[p_nb][h]
                            nc.tensor.matmul(
                                pv,
                                lhsT=V1[:, 2 * p_pair:2 * p_pair + 2, h, 0:65],
                                rhs=p_pexp[:, :, h * 512:(h + 1) * 512],
                                start=(p_pair == 0),
                                stop=(p_pair == MB // 2 - 1),
                                perf_mode=mybir.MatmulPerfMode.DoubleRow,
                                skip_group_check=True)
                            if p_pair == MB // 2 - 1:
                                # Release this pv PSUM bank with a single
                                # [65,512] ScalarE copy: rows 0:64 raw A,
                                # row 64 the denominator (same per-lane cost
                                # as copying A alone; no DVE involvement).
                                pv2 = pv_tiles[p_nb][h]
                                pvs = denp.tile([65, 512], F32, tag="den")
                                nc.scalar.activation(
                                    pvs, pv2,
                                    mybir.ActivationFunctionType.Identity,
                                    scale=1.0)
                                recs.setdefault(p_nb, []).append(pvs)

                        # PV-lo consumes the ScalarE halves (early) in this
                        # iteration; PV-hi consumes the DVE halves one
                        # iteration later, so the in-order PE queue never
                        # blocks on the lagging Schraudolph write.
                        if pending_pv is not None:
                            emit_pv(*pending_pv, 1)
                            pending_pv = None
                        if ko == 1:
                            emit_pv(nb, mb // 2, pexp_cur, 0)
                            pending_pv = (nb, mb // 2, pexp_cur)

                    # flush the final PV-hi + the last block's normalize/scale
                    # (inside the pool scope)
                    emit_pv(*pending_pv, 1)
                    for h2 in range(2):
                        emit_normalize(NB - 1, h2)
                        emit_scale(NB - 1, h2)

                # ---- proj tail: runs after the attention pools close, in
                # the PSUM banks they freed ----
                with tc.tile_pool(name="ps_pj", bufs=2, space="PSUM") as ps_pj:
                    for nbp in range(NB):
                        ns = slice(nbp * 512, (nbp + 1) * 512)
                        for m in range(2):
                            ps_y = ps_pj.tile([128, 512], F32, tag="pj")
                            nc.tensor.matmul(ps_y, lhsT=wp_lo[:, m * 128:(m + 1) * 128],
                                             rhs=A_lo[:, ns], start=True, stop=False)
                            nc.tensor.matmul(ps_y, lhsT=wp_hi[:, m * 128:(m + 1) * 128],
                                             rhs=A_hi[:, ns], start=False, stop=True)
                            y_sb = yout.tile([128, 512], F32, tag="y")
                            if m == 0:
                                nc.scalar.activation(
                                    y_sb, ps_y,
                                    mybir.ActivationFunctionType.Identity,
                                    scale=1.0)
                            else:
                                nc.vector.tensor_copy(y_sb, ps_y)
                            nc.sync.dma_start(out=yp[m * 128:(m + 1) * 128, ns],
                                              in_=y_sb)

    nc.finalize()
    return nc


_CACHE = {}


ATTN_DTYPE = BF16
MM_DTYPE = BF16


def _get_program(has_v_bias: bool, chain: int = 1):
    key = ("prog", has_v_bias, str(ATTN_DTYPE), str(MM_DTYPE), chain)
    if key not in _CACHE:
        _CACHE[key] = _build_program(has_v_bias, ATTN_DTYPE, MM_DTYPE, chain)
    return _CACHE[key]


def _make_in_maps(x, gn_w, gn_b, qkv_w, qkv_b, proj_w):
    x = np.ascontiguousarray(x, dtype=np.float32)
    in_maps = []
    for core in range(NCORES):
        b, p = core // 2, core % 2
        rows_q = slice(p * 128, (p + 1) * 128)
        rows_k = slice(256 + p * 128, 256 + (p + 1) * 128)
        rows_v = slice(512 + p * 128, 512 + (p + 1) * 128)
        m = {
            "xb": np.ascontiguousarray(x[b].reshape(C, N)),
            "wqkT": np.ascontiguousarray(
                np.concatenate([qkv_w[rows_q] * LOG2E, qkv_w[rows_k]],
                               axis=0).T.astype(np.float32)),
            "wvT": np.ascontiguousarray(qkv_w[rows_v].T.astype(np.float32)),
            "wpT": np.ascontiguousarray(proj_w[:, p * 128:(p + 1) * 128].T.astype(np.float32)),
            "gnw": np.ascontiguousarray(gn_w.astype(np.float32)),
            "gnb": np.ascontiguousarray(gn_b.astype(np.float32)),
            "qkb": np.ascontiguousarray(
                np.concatenate([qkv_b[rows_q] * LOG2E,
                                qkv_b[rows_k]]).astype(np.float32)),
        }
        if np.any(qkv_b[512:768]):
            m["vb"] = np.ascontiguousarray(qkv_b[rows_v].astype(np.float32))
        in_maps.append(m)
    return in_maps


def _get_executor(nc, donate=True):
    """Build (once) a cached jitted 8-core executor for the program.

    Mirrors concourse.bass2jax.run_bass_via_pjrt, but caches the jitted
    callable so repeat kernel() calls don't re-trace/re-compile the XLA
    wrapper.  Returns (fn, in_names, out_names) where fn takes a list of
    per-core input dicts and returns a list of per-core output dicts.
    """
    key = ("exec", id(nc), donate)
    if key in _CACHE:
        return _CACHE[key]
    import jax
    import concourse.mybir as _mybir
    from jax.experimental.shard_map import shard_map
    from jax.sharding import Mesh, PartitionSpec
    from concourse import bass2jax

    bass2jax.install_neuronx_cc_hook()
    partition_name = nc.partition_id_tensor.name if nc.partition_id_tensor else None
    in_names, out_names, out_avals, zero_outs = [], [], [], []
    for alloc in nc.m.functions[0].allocations:
        if not isinstance(alloc, _mybir.MemoryLocationSet):
            continue
        name = alloc.memorylocations[0].name
        if alloc.kind == "ExternalInput":
            if name != partition_name:
                in_names.append(name)
        elif alloc.kind == "ExternalOutput":
            shape = tuple(alloc.tensor_shape)
            dtype = _mybir.dt.np(alloc.dtype)
            out_names.append(name)
            out_avals.append(jax.core.ShapedArray(shape, dtype))
            zero_outs.append(np.zeros(shape, dtype))
    n_params = len(in_names)
    n_outs = len(out_avals)
    all_names = in_names + out_names + ([partition_name] if partition_name else [])

    def _body(*args):
        operands = list(args)
        if partition_name is not None:
            operands.append(bass2jax.partition_id_tensor())
        return tuple(bass2jax._bass_exec_p.bind(
            *operands,
            out_avals=tuple(out_avals),
            in_names=tuple(all_names),
            out_names=tuple(out_names),
            lowering_input_output_aliases=(),
            sim_require_finite=True,
            sim_require_nnan=True,
            nc=nc,
        ))

    devices = jax.devices()[:NCORES]
    mesh = Mesh(np.asarray(devices), ("core",))
    in_specs = (PartitionSpec("core"),) * (n_params + n_outs)
    out_specs = (PartitionSpec("core"),) * n_outs
    donate_idx = tuple(range(n_params, n_params + n_outs)) if donate else ()
    sharded = jax.jit(
        shard_map(_body, mesh=mesh, in_specs=in_specs, out_specs=out_specs,
                  check_rep=False),
        donate_argnums=donate_idx, keep_unused=True,
    )

    _CACHE[("sharded", id(nc))] = sharded
    _CACHE[("zeros", id(nc))] = [((NCORES * z.shape[0],) + z.shape[1:], z.dtype)
                                 for z in zero_outs]

    def fn(in_maps):
        concat_in = [
            np.concatenate([np.asarray(in_maps[c][nm]) for c in range(NCORES)], axis=0)
            for nm in in_names
        ]
        concat_zeros = [
            np.zeros((NCORES * z.shape[0], *z.shape[1:]), z.dtype) for z in zero_outs
        ]
        out_arrs = sharded(*concat_in, *concat_zeros)
        return [
            {nm: np.asarray(out_arrs[i]).reshape(NCORES, *out_avals[i].shape)[c]
             for i, nm in enumerate(out_names)}
            for c in range(NCORES)
        ]

    _CACHE[key] = (fn, in_names, out_names)
    return _CACHE[key]


def _prep(inputs):
    x = np.asarray(inputs["x"], dtype=np.float32)
    qkv_b = np.asarray(inputs["qkv_b"], dtype=np.float32)
    has_v_bias = bool(np.any(qkv_b[512:768]))
    nc = _get_program(has_v_bias)
    in_maps = _make_in_maps(
        x,
        np.asarray(inputs["gn_w"], dtype=np.float32),
        np.asarray(inputs["gn_b"], dtype=np.float32),
        np.asarray(inputs["qkv_w"], dtype=np.float32),
        qkv_b,
        np.asarray(inputs["proj_w"], dtype=np.float32),
    )
    return nc, in_maps, x


def run(inputs, trace=False):
    """Run the sharded kernel.  Returns (output, per-core results list)."""
    nc, in_maps, x = _prep(inputs)
    fn, _, _ = _get_executor(nc)
    results = fn(in_maps)
    proj_b = np.asarray(inputs["proj_b"], dtype=np.float32)
    parts = [results[c]["yp"] for c in range(NCORES)]
    y = np.stack([parts[2 * b] + parts[2 * b + 1] for b in range(B)])  # [B, C, N]
    y = y + proj_b[None, :, None]
    out = np.asarray(inputs["x"], dtype=np.float32) + y.reshape(B, C, 64, 64)
    return out.astype(np.float32), results


def kernel(**inputs) -> np.ndarray:
    out, _ = run(inputs, trace=False)
    return out



# revision 72
# speedup vs baseline: 1.0241x; 1.0109x over previous
"""Trainium2 Bass kernel for nn_AttentionBlock (GroupNorm + 4-head attention + proj + residual).

Sharding: 8 cores = (batch b in 0..3) x (head-pair p in 0..1).  Each core gets
x[b] and the weight slices for its two heads, computes GroupNorm -> QKV ->
attention -> partial proj (over its 128 attention-output channels), and returns
a partial [256, 4096] projection output.  The host sums the two partials per
batch, adds proj bias + residual, and reshapes.

The device program is identical on all cores (SPMD); all per-core variation is
carried by the input tensors.

Layout notes (per core):
  h   = groupnorm(x[b])                SBUF [c=128 x2, n=4096] bf16
  Q2  = log2e * (Wq_pair^T . h + bq)   SBUF [128 (2 heads x 64 d), 4096] bf16
  K2  = Wk_pair^T . h + bk             SBUF [128, 4096] bf16
  V1  = [h^T . Wv_pair | ones]         SBUF [128 (m-chunk), 32, 2, 80] fp8e4
  scores'^T[m, n] = sum_d K[d,m] Q[d,n] via matmul(lhsT=K2[64 rows], rhs=Q2),
     two heads packed concurrently in PE row-groups (0,0) and (64,0);
     = log2e * raw scores thanks to the host-side Q pre-scale.
  pexp (fp8e4, ~exp(0.125*raw)*2^(-C/8)) is produced half-and-half:
     ScalarE: exact Exp with free affine (scale 0.125/log2e, bias -C/8*ln2)
     DVE:     Schraudolph bits8 = clamp(qk' + 56 - C) via one tensor_scalar
              (max, add) written through an int8 view of the fp8 tile.
     NB fp8e4 here is IEEE-style: exponent 15 = Inf/NaN, so C keeps bits <= ~116.
  pv[d+1, n] += V1^T . pexp            fp8 DoubleRow matmuls, each contracting
     a PAIR of 128-key chunks (lhsT [128,2,65], rhs [128,2,512]); row 64
     (the ones column) accumulates the softmax denominator.
  At the end of each n-block: den rows copied out on the DVE, raw A on
  ScalarE (frees the PSUM banks fast), then off the critical path:
  DRAM-bounce broadcast of den, approx-reciprocal on the [128,512] broadcast
  (base-partition 0 only - the custom-DVE op breaks at other bases), A *= 1/den,
  proj accumulates both heads into one PSUM tile per m.
  y_partial = Wp[:, pair]^T . A        [256, 4096] -> DRAM
"""

import numpy as np

import concourse.bacc as bacc
import concourse.bass as bass

import concourse.mybir as mybir
import concourse.tile as tile

B = 4
C = 256
N = 4096          # 64*64
NH = 4
D = 64            # head dim
GROUPS = 32
EPS = 1e-5
NCORES = 8
SCALE = float(D) ** -0.5  # 0.125
F32 = mybir.dt.float32
F32R = mybir.dt.float32r
BF16 = mybir.dt.bfloat16
F8 = mybir.dt.float8e4
I8 = mybir.dt.int8

MB = 32           # m chunks of 128
NB = 8            # n chunks of 512

# fp8-e4m3 softmax weights + DoubleRow PV.
#
# The Q weights/bias are pre-scaled by log2(e) on the host, so the QK matmul
# emits qk' = log2e * qk_raw, i.e. 8*t where exp(0.125*qk_raw) = 2^t.
#  - ScalarE half (head-lo): exact exp -> fp8e4 out, scaled by 2^(-C/8) via
#    the activation's free affine (scale = 0.125/log2e, bias = -C/8*ln2).
#  - DVE half (head-hi): Schraudolph bits8 = clamp(qk' + 56 - C, >=0) via one
#    tensor_scalar (op0 = max, op1 = add) written through an int8 view.
# The 2^(-C/8) factor cancels in the softmax normalization.  PV runs fp8
# DoubleRow: each matmul contracts TWO 128-key chunks at once (rhs [128,2,512],
# lhsT [128,2,65]), halving the PV instruction count.
LOG2E = 1.4426950408889634
LN2 = 0.6931471805599453
F8C = 24.0                      # exponent offset: bits8 <= ~116 (qk' max ~84); fp8e4 here is IEEE-style, exponent 15 (bits >= 120) = Inf/NaN
EXPSC = 0.125 / LOG2E           # ACT exp input scale (qk' -> 0.125*qk_raw)
EXPB = -(F8C / 8.0) * LN2       # ACT exp input bias  (output *= 2^(-C/8))
SCHROFF = 56.0 - F8C            # DVE: bits8 = max(qk', -SCHROFF) + SCHROFF


def _build_program(has_v_bias: bool, attn_dtype=BF16, mm_dtype=BF16, chain: int = 1):
    nc = bacc.Bacc("TRN2", target_bir_lowering=False)
    MMD = mm_dtype        # dtype of tiles feeding qkv/proj matmuls (h, weights, A)
    MAD = attn_dtype      # dtype of lhsT tiles for QK/PV (K2, V1)
    MADR = attn_dtype     # dtype of rhs tiles for QK/PV (Q2, pexp)

    xb = nc.dram_tensor("xb", [C, N], F32, kind="ExternalInput")
    wqkT = nc.dram_tensor("wqkT", [C, 256], F32, kind="ExternalInput")   # cols 0:128 Qpair, 128:256 Kpair
    wvT = nc.dram_tensor("wvT", [C, 128], F32, kind="ExternalInput")
    wpT = nc.dram_tensor("wpT", [128, C], F32, kind="ExternalInput")     # rows = pair channels
    gnw = nc.dram_tensor("gnw", [C], F32, kind="ExternalInput")
    gnb = nc.dram_tensor("gnb", [C], F32, kind="ExternalInput")
    qkb = nc.dram_tensor("qkb", [256], F32, kind="ExternalInput")        # 0:128 Q bias pair, 128:256 K bias pair
    if has_v_bias:
        vb = nc.dram_tensor("vb", [128], F32, kind="ExternalInput")
    yp = nc.dram_tensor("yp", [C, N], F32, kind="ExternalOutput")

    # group-indicator matrix: G[c, g] = 1 if c // 8 == g else 0  (per c-tile)
    g_host = np.zeros((128, 16), dtype=np.float32)
    for c in range(128):
        g_host[c, c // 8] = 1.0
    g_dram = nc.inline_tensor(g_host, name="gmat")
    gt_dram = nc.inline_tensor(np.ascontiguousarray(g_host.T), name="gmatT")

    with tile.TileContext(nc) as tc:
        with (
            tc.tile_pool(name="consts", bufs=1) as consts,
            tc.tile_pool(name="xh", bufs=2) as xh_pool,
            tc.tile_pool(name="hp", bufs=2) as hp_pool,
            tc.tile_pool(name="qk_sb", bufs=1) as qk_sb,
            tc.tile_pool(name="v1p", bufs=1) as v1p,
            tc.tile_pool(name="st", bufs=2) as st,
            tc.tile_pool(name="pexpa", bufs=6) as pexpa,
            tc.tile_pool(name="denp", bufs=4) as denp,
            tc.tile_pool(name="bcp", bufs=8) as bcp,
            tc.tile_pool(name="drec", bufs=4, space="DRAM") as drec,
            tc.tile_pool(name="yout", bufs=4) as yout,
        ):
            # ---- constants ----
            g_sb = consts.tile([128, 16], F32, tag="gsb")
            nc.sync.dma_start(out=g_sb, in_=g_dram[:, :])
            gt_sb = consts.tile([16, 128], F32, tag="gtsb")
            nc.sync.dma_start(out=gt_sb, in_=gt_dram[:, :])

            def load_weight(shape, dram_slice, tag):
                w_raw = consts.tile(shape, F32, tag=tag + "_r", name=tag + "_r")
                nc.sync.dma_start(out=w_raw, in_=dram_slice)
                if MMD == F32:
                    return w_raw
                w_c = consts.tile(shape, MMD, tag=tag, name=tag)
                nc.vector.tensor_copy(w_c, w_raw)
                return w_c

            w_qk = [load_weight([128, 256], wqkT[t * 128:(t + 1) * 128, :], f"wqk{t}")
                    for t in range(2)]
            w_v = [load_weight([128, 128], wvT[t * 128:(t + 1) * 128, :], f"wv{t}")
                   for t in range(2)]
            wp_lo = load_weight([64, 256], wpT[0:64, :], "wplo")
            wp_hi = load_weight([64, 256], wpT[64:128, :], "wphi")

            gw_t, gb_t = [], []
            for t in range(2):
                gwt = consts.tile([128, 1], F32, tag=f"gw{t}")
                nc.sync.dma_start(out=gwt, in_=gnw[t * 128:(t + 1) * 128])
                gw_t.append(gwt)
                gbt = consts.tile([128, 1], F32, tag=f"gb{t}")
                nc.sync.dma_start(out=gbt, in_=gnb[t * 128:(t + 1) * 128])
                gb_t.append(gbt)
            bq = consts.tile([128, 1], F32, tag="bq")
            nc.sync.dma_start(out=bq, in_=qkb[0:128])
            bk = consts.tile([128, 1], F32, tag="bk")
            nc.sync.dma_start(out=bk, in_=qkb[128:256])
            if has_v_bias:
                vb_sb = consts.tile([128, 128], F32, tag="vbsb")
                nc.sync.dma_start(
                    out=vb_sb,
                    in_=bass.AP(tensor=vb, offset=0, ap=[[0, 128], [1, 128]]),
                )
            eps16 = consts.tile([16, 1], F32, tag="eps16")
            nc.vector.memset(eps16, EPS)
            eighth = consts.tile([16, 1], F32, tag="eighth")
            nc.vector.memset(eighth, 0.125)
            expb = consts.tile([128, 1], F32, tag="expb")
            nc.vector.memset(expb, EXPB)

            for _rep in range(chain):
                # ---- phase A: GroupNorm stats ----
                h_t = []
                scale_t, bias_t = [], []
                with (
                    tc.tile_pool(name="ps_g", bufs=2, space="PSUM") as ps_g,
                    tc.tile_pool(name="ps_warm", bufs=2, space="PSUM") as ps_warm,
                ):
                    # dummy Ln: preloads the natural_log act table while the
                    # x DMA is still in flight.
                    dummy = st.tile([16, 1], F32, tag="dummy")
                    nc.scalar.activation(dummy, eps16,
                                         mybir.ActivationFunctionType.Ln, scale=1.0)
                    x_t_list = []
                    gsb_ts, var_ts = [], []
                    for t in range(2):
                        x_t = xh_pool.tile([128, N], F32, tag="x", name=f"x{t}")
                        stats = st.tile([128, 8, 6], F32, tag="stats")
                        xr = x_t.rearrange("p (k f) -> p k f", f=512)
                        # 8 x 256KB DMAs per tile: each dma_start lands on
                        # ONE DMA engine (~22 GB/s), so a 512KB quarter took
                        # ~23 us serial - smaller chunks spread over all 16
                        # engines cut the x-load wall to the ~11 us aggregate
                        # bandwidth floor.
                        for k in range(8):
                            ks = slice(k * 512, (k + 1) * 512)
                            nc.sync.dma_start(out=x_t[:, ks],
                                              in_=xb[t * 128:(t + 1) * 128, ks])
                            if k % 2 == 0:
                                # junk matmul on the freshly-landed chunk:
                                # keeps the PE HAM activity monitor warm
                                # through the DMA/stats prologue.
                                wj = ps_warm.tile([16, 512], F32, tag="warm")
                                nc.tensor.matmul(wj, lhsT=g_sb,
                                                 rhs=xr[:, k, :],
                                                 start=True, stop=True)
                            # stats per landed chunk - overlaps the DMA
                            nc.vector.bn_stats(out=stats[:, k, :],
                                               in_=xr[:, k, :])
                        x_t_list.append(x_t)
                        mv = st.tile([128, 2], F32, tag="mv")
                        nc.vector.bn_aggr(out=mv, in_=stats)

                        # mq = [mean_c, var_c + mean_c^2]
                        mq = st.tile([128, 2], F32, tag="mq")
                        nc.vector.tensor_copy(mq[:, 0:1], mv[:, 0:1])
                        sq = st.tile([128, 1], F32, tag="sq")
                        nc.vector.tensor_mul(sq, mv[:, 0:1], mv[:, 0:1])
                        nc.vector.tensor_add(mq[:, 1:2], mv[:, 1:2], sq)

                        gps = ps_g.tile([16, 2], F32, tag="gps")
                        nc.tensor.matmul(gps, lhsT=g_sb, rhs=mq, start=True, stop=True)
                        # per-group E[x], E[x^2]
                        gsb = st.tile([16, 2], F32, tag=f"gsb2_{t}")
                        nc.vector.tensor_mul(gsb, gps, eighth.to_broadcast((16, 2)))
                        sqg = st.tile([16, 1], F32, tag="sqg")
                        nc.vector.tensor_mul(sqg, gsb[:, 0:1], gsb[:, 0:1])
                        var = st.tile([16, 1], F32, tag=f"var{t}")
                        nc.vector.tensor_sub(var, gsb[:, 1:2], sqg)
                        gsb_ts.append(gsb)
                        var_ts.append(var)

                    # rstd = exp(-0.5 * ln(var + eps)), grouped Ln,Ln / Exp,Exp
                    # so ScalarE loads each act table at most once.
                    lnv_ts = []
                    for t in range(2):
                        lnv = st.tile([16, 1], F32, tag=f"lnv{t}")
                        nc.scalar.activation(lnv, var_ts[t],
                                             mybir.ActivationFunctionType.Ln,
                                             bias=eps16, scale=1.0)
                        lnv_ts.append(lnv)
                    rstd_ts = []
                    for t in range(2):
                        rstd = st.tile([16, 1], F32, tag=f"rstd{t}")
                        nc.scalar.activation(rstd, lnv_ts[t],
                                             mybir.ActivationFunctionType.Exp,
                                             scale=-0.5)
                        rstd_ts.append(rstd)

                    for t in range(2):
                        ms = st.tile([16, 2], F32, tag=f"ms{t}")
                        nc.vector.tensor_copy(ms[:, 0:1], gsb_ts[t][:, 0:1])
                        nc.vector.tensor_copy(ms[:, 1:2], rstd_ts[t])

                        # broadcast per-group stats to per-channel [128,2] via
                        # a tiny PE matmul with the transposed group indicator.
                        bps = ps_g.tile([128, 2], F32, tag="bps")
                        nc.tensor.matmul(bps, lhsT=gt_sb, rhs=ms, start=True, stop=True)
                        sc = consts.tile([128, 1], F32, tag=f"scale{t}")
                        nc.vector.tensor_mul(sc, bps[:, 1:2], gw_t[t])
                        scale_t.append(sc)
                        tmp = st.tile([128, 1], F32, tag="tmpb")
                        nc.vector.tensor_mul(tmp, bps[:, 0:1], sc)
                        bi = consts.tile([128, 1], F32, tag=f"bias{t}")
                        nc.vector.tensor_sub(bi, gb_t[t], tmp)
                        bias_t.append(bi)

                # ---- phase B: GroupNorm h + QKV, interleaved per 512-column
                # chunk so the PE never waits multiple microseconds behind a
                # full-width DVE pass (which used to re-throttle the HAM).
                Q2 = qk_sb.tile([128, N], MADR, tag="q2")
                K2 = qk_sb.tile([128, N], MAD, tag="k2")
                # fp8 V^T, padded to 80 cols so the DoubleRow ko-stride (80 B)
                # is a multiple of 16
                V1 = v1p.tile([128, MB, 2, 80], F8, tag="v1")
                for t in range(2):
                    h_new = hp_pool.tile([128, N], MMD, tag="h", name=f"h{t}")
                    h_t.append(h_new)
                with tc.tile_pool(name="ps_qkv", bufs=3, space="PSUM") as ps_qkv:
                    for nb in range(NB):
                        ns = slice(nb * 512, (nb + 1) * 512)
                        for t in range(2):
                            # h = x * scale + bias (one 512-col chunk)
                            nc.vector.tensor_scalar(
                                out=h_t[t][:, ns], in0=x_t_list[t][:, ns],
                                scalar1=scale_t[t], scalar2=bias_t[t],
                                op0=mybir.AluOpType.mult, op1=mybir.AluOpType.add,
                            )
                        if nb == 0:
                            nc.vector.memset(V1, 1.0)
                        for dst, col0, bias_ap in ((Q2, 0, bq), (K2, 128, bk)):
                            ps = ps_qkv.tile([128, 512], F32, tag="mm")
                            nc.tensor.matmul(ps, lhsT=w_qk[0][:, col0:col0 + 128],
                                             rhs=h_t[0][:, ns], start=True, stop=False)
                            nc.tensor.matmul(ps, lhsT=w_qk[1][:, col0:col0 + 128],
                                             rhs=h_t[1][:, ns], start=False, stop=True)
                            # bias-add on the otherwise-idle ScalarE (Identity
                            # is in every act table set - no table reload).
                            nc.scalar.activation(
                                dst[:, ns], ps, mybir.ActivationFunctionType.Identity,
                                bias=bias_ap, scale=1.0)
                        # V^T chunks for this 512-col chunk (frees the proj
                        # PSUM banks for a 3-deep qk pipeline in phase C)
                        for q4 in range(4):
                            mbv = nb * 4 + q4
                            psv = ps_qkv.tile([128, 128], F32, tag="mm",
                                              name=f"psv_{mbv}")
                            cs = slice(mbv * 128, (mbv + 1) * 128)
                            nc.tensor.matmul(psv, lhsT=h_t[0][:, cs],
                                             rhs=w_v[0], start=True, stop=False)
                            nc.tensor.matmul(psv, lhsT=h_t[1][:, cs],
                                             rhs=w_v[1], start=False, stop=True)
                            if has_v_bias:
                                nc.vector.tensor_add(
                                    V1[:, mbv, :, 0:64],
                                    psv.rearrange("p (h d) -> p h d", h=2),
                                    vb_sb.rearrange("p (h d) -> p h d", h=2),
                                )
                            else:
                                nc.vector.tensor_copy(
                                    V1[:, mbv, :, 0:64],
                                    psv.rearrange("p (h d) -> p h d", h=2),
                                )

                # ---- phase C: attention ----
                A_lo = xh_pool.tile([64, N], MMD, tag="x", name="A_lo")
                A_hi = xh_pool.tile([64, N], MMD, tag="x", name="A_hi")
                with (
                    tc.tile_pool(name="ps_qk", bufs=3, space="PSUM") as ps_qk,
                    tc.tile_pool(name="ps_pv", bufs=2, space="PSUM") as ps_pv,
                ):
                    recs, bcs = {}, {}

                    def emit_normalize(nb2, h2):
                        # broadcast the raw denominator across all partitions
                        # by bouncing through DRAM (a DRAM AP may have a
                        # zero-step partition dim; SBUF cannot), then take the
                        # fast approx reciprocal on the [128, 512] broadcast.
                        # (reciprocal_approx_fast is base-partition-0 only: at
                        # other bases the custom-DVE op reads garbage on HW.)
                        pvs = recs[nb2][h2]
                        dr = drec.tile([1, 512], F32, tag="dr")
                        nc.sync.dma_start(out=dr, in_=pvs[64:65, :])
                        bcr = bcp.tile([128, 512], F32, tag="bcr")
                        nc.sync.dma_start(
                            out=bcr,
                            in_=bass.AP(tensor=dr.tensor, offset=dr.offset,
                                        ap=[[0, 128]] + list(dr.ap[1:])),
                        )
                        bc = bcp.tile([128, 512], F32, tag="bc")
                        nc.vector.reciprocal_approx_fast(out=bc, in_=bcr)
                        bcs[(nb2, h2)] = (bc, pvs)

                    def emit_scale(nb, h2):
                        # A = raw_pv * 1/den ([64, 512] per head - half the
                        # elements of scaling the proj outputs); off the PE
                        # critical path, the bc broadcast had a head start.
                        ns = slice(nb * 512, (nb + 1) * 512)
                        bc, pvs = bcs.pop((nb, h2))
                        A = A_lo if h2 == 0 else A_hi
                        nc.vector.tensor_mul(A[:, ns], pvs[0:64, :], bc[0:64, :])

                    def emit_qk(nb, mb):
                        ns = slice(nb * 512, (nb + 1) * 512)
                        ms_ = slice(mb * 128, (mb + 1) * 128)
                        qk = ps_qk.tile([128, 1024], F32, tag="qk", name=f"qk_{nb}_{mb}")
                        nc.tensor.matmul(qk[:, 0:512], lhsT=K2[0:64, ms_],
                                         rhs=Q2[0:64, ns], start=True, stop=True,
                                         skip_group_check=True)
                        nc.tensor.matmul(qk[:, 512:1024], lhsT=K2[64:128, ms_],
                                         rhs=Q2[64:128, ns], start=True, stop=True,
                                         skip_group_check=True)
                        return qk

                    # software pipeline: emit iteration i+1's QK matmuls before
                    # iteration i's PV matmuls, so the in-order PE queue never
                    # stalls behind a PV that waits on ScalarE's exp.
                    iters = [(nb, mb) for nb in range(NB) for mb in range(MB)]
                    pv_tiles = {}
                    pending_pv = None
                    qk_fifo = [emit_qk(*iters[0]), emit_qk(*iters[1]),
                               emit_qk(*iters[2])]
                    for idx, (nb, mb) in enumerate(iters):
                        ns = slice(nb * 512, (nb + 1) * 512)
                        if nb > 0 and mb in (6, 10, 14, 18):
                            # spread the per-head normalize/scale ops out so
                            # they never bunch up in the DVE queue
                            (emit_normalize if mb < 14 else emit_scale)(
                                nb - 1, 0 if mb in (6, 14) else 1)
                        if mb == 0:
                            pv_lo = ps_pv.tile([65, 512], F32, tag="pv", name=f"pvlo_{nb}")
                            pv_hi = ps_pv.tile([65, 512], F32, tag="pv", name=f"pvhi_{nb}")
                            pv_tiles[nb] = (pv_lo, pv_hi)
                        pv_lo, pv_hi = pv_tiles[nb]
                        qk_cur = qk_fifo.pop(0)
                        ko = mb % 2
                        if ko == 0:
                            pexp_cur = pexpa.tile([128, 2, 1024], F8, tag="pexpa",
                                                  name=f"pexp_{nb}_{mb}")
                        # exp split across both engines: exact exp of the lo
                        # half on ScalarE (fp8e4 out), Schraudolph bits of the
                        # hi half on the DVE through an int8 view.
                        nc.scalar.activation(pexp_cur[:, ko, 0:512],
                                             qk_cur[:, 0:512],
                                             mybir.ActivationFunctionType.Exp,
                                             scale=EXPSC, bias=expb)
                        nc.vector.tensor_scalar(
                            out=pexp_cur.bitcast(I8)[:, ko, 512:1024],
                            in0=qk_cur[:, 512:1024],
                            scalar1=-SCHROFF, scalar2=SCHROFF,
                            op0=mybir.AluOpType.max, op1=mybir.AluOpType.add,
                        )
                        # 3-deep QK pipeline: QK_{i+3} reuses qk_i's PSUM
                        # banks, so two QKs can run during one exp and the
                        # engines' exps go back-to-back.
                        if idx + 3 < len(iters):
                            qk_fifo.append(emit_qk(*iters[idx + 3]))
                        def emit_pv(p_nb, p_pair, p_pexp, h):
                            # DoubleRow PV: one matmul per head contracts the
                            # chunk PAIR (2 x 128 keys) at once.
                            pv = pv_tiles[p_nb][h]
                            nc.tensor.matmul(
                                pv,
                                lhsT=V1[:, 2 * p_pair:2 * p_pair + 2, h, 0:65],
                                rhs=p_pexp[:, :, h * 512:(h + 1) * 512],
                                start=(p_pair == 0),
                                stop=(p_pair == MB // 2 - 1),
                                perf_mode=mybir.MatmulPerfMode.DoubleRow,
                                skip_group_check=True)
                            if p_pair == MB // 2 - 1:
                                # Release this pv PSUM bank with a single
                                # [65,512] ScalarE copy: rows 0:64 raw A,
                                # row 64 the denominator (same per-lane cost
                                # as copying A alone; no DVE involvement).
                                pv2 = pv_tiles[p_nb][h]
                                pvs = denp.tile([65, 512], F32, tag="den")
                                nc.scalar.activation(
                                    pvs, pv2,
                                    mybir.ActivationFunctionType.Identity,
                                    scale=1.0)
                                recs.setdefault(p_nb, []).append(pvs)

                        # PV-lo consumes the ScalarE halves (early) in this
                        # iteration; PV-hi consumes the DVE halves one
                        # iteration later, so the in-order PE queue never
                        # blocks on the lagging Schraudolph write.
                        if pending_pv is not None:
                            emit_pv(*pending_pv, 1)
                            pending_pv = None
                        if ko == 1:
                            emit_pv(nb, mb // 2, pexp_cur, 0)
                            pending_pv = (nb, mb // 2, pexp_cur)

                    # flush the final PV-hi + the last block's normalize/scale
                    # (inside the pool scope)
                    emit_pv(*pending_pv, 1)
                    for h2 in range(2):
                        emit_normalize(NB - 1, h2)
                        emit_scale(NB - 1, h2)

                # ---- proj tail: runs after the attention pools close, in
                # the PSUM banks they freed ----
                with tc.tile_pool(name="ps_pj", bufs=2, space="PSUM") as ps_pj:
                    for nbp in range(NB):
                        ns = slice(nbp * 512, (nbp + 1) * 512)
                        for m in range(2):
                            ps_y = ps_pj.tile([128, 512], F32, tag="pj")
                            nc.tensor.matmul(ps_y, lhsT=wp_lo[:, m * 128:(m + 1) * 128],
                                             rhs=A_lo[:, ns], start=True, stop=False)
                            nc.tensor.matmul(ps_y, lhsT=wp_hi[:, m * 128:(m + 1) * 128],
                                             rhs=A_hi[:, ns], start=False, stop=True)
                            y_sb = yout.tile([128, 512], F32, tag="y")
                            if m == 0:
                                nc.scalar.activation(
                                    y_sb, ps_y,
                                    mybir.ActivationFunctionType.Identity,
                                    scale=1.0)
                            else:
                                nc.vector.tensor_copy(y_sb, ps_y)
                            nc.sync.dma_start(out=yp[m * 128:(m + 1) * 128, ns],
                                              in_=y_sb)

    nc.finalize()
    return nc


_CACHE = {}


ATTN_DTYPE = BF16
MM_DTYPE = BF16


def _get_program(has_v_bias: bool, chain: int = 1):
    key = ("prog", has_v_bias, str(ATTN_DTYPE), str(MM_DTYPE), chain)
    if key not in _CACHE:
        _CACHE[key] = _build_program(has_v_bias, ATTN_DTYPE, MM_DTYPE, chain)
    return _CACHE[key]


def _make_in_maps(x, gn_w, gn_b, qkv_w, qkv_b, proj_w):
    x = np.ascontiguousarray(x, dtype=np.float32)
    in_maps = []
    for core in range(NCORES):
        b, p = core // 2, core % 2
        rows_q = slice(p * 128, (p + 1) * 128)
        rows_k = slice(256 + p * 128, 256 + (p + 1) * 128)
        rows_v = slice(512 + p * 128, 512 + (p + 1) * 128)
        m = {
            "xb": np.ascontiguousarray(x[b].reshape(C, N)),
            "wqkT": np.ascontiguousarray(
                np.concatenate([qkv_w[rows_q] * LOG2E, qkv_w[rows_k]],
                               axis=0).T.astype(np.float32)),
            "wvT": np.ascontiguousarray(qkv_w[rows_v].T.astype(np.float32)),
            "wpT": np.ascontiguousarray(proj_w[:, p * 128:(p + 1) * 128].T.astype(np.float32)),
            "gnw": np.ascontiguousarray(gn_w.astype(np.float32)),
            "gnb": np.ascontiguousarray(gn_b.astype(np.float32)),
            "qkb": np.ascontiguousarray(
                np.concatenate([qkv_b[rows_q] * LOG2E,
                                qkv_b[rows_k]]).astype(np.float32)),
        }
        if np.any(qkv_b[512:768]):
            m["vb"] = np.ascontiguousarray(qkv_b[rows_v].astype(np.float32))
        in_maps.append(m)
    return in_maps


def _get_executor(nc, donate=True):
    """Build (once) a cached jitted 8-core executor for the program.

    Mirrors concourse.bass2jax.run_bass_via_pjrt, but caches the jitted
    callable so repeat kernel() calls don't re-trace/re-compile the XLA
    wrapper.  Returns (fn, in_names, out_names) where fn takes a list of
    per-core input dicts and returns a list of per-core output dicts.
    """
    key = ("exec", id(nc), donate)
    if key in _CACHE:
        return _CACHE[key]
    import jax
    import concourse.mybir as _mybir
    from jax.experimental.shard_map import shard_map
    from jax.sharding import Mesh, PartitionSpec
    from concourse import bass2jax

    bass2jax.install_neuronx_cc_hook()
    partition_name = nc.partition_id_tensor.name if nc.partition_id_tensor else None
    in_names, out_names, out_avals, zero_outs = [], [], [], []
    for alloc in nc.m.functions[0].allocations:
        if not isinstance(alloc, _mybir.MemoryLocationSet):
            continue
        name = alloc.memorylocations[0].name
        if alloc.kind == "ExternalInput":
            if name != partition_name:
                in_names.append(name)
        elif alloc.kind == "ExternalOutput":
            shape = tuple(alloc.tensor_shape)
            dtype = _mybir.dt.np(alloc.dtype)
            out_names.append(name)
            out_avals.append(jax.core.ShapedArray(shape, dtype))
            zero_outs.append(np.zeros(shape, dtype))
    n_params = len(in_names)
    n_outs = len(out_avals)
    all_names = in_names + out_names + ([partition_name] if partition_name else [])

    def _body(*args):
        operands = list(args)
        if partition_name is not None:
            operands.append(bass2jax.partition_id_tensor())
        return tuple(bass2jax._bass_exec_p.bind(
            *operands,
            out_avals=tuple(out_avals),
            in_names=tuple(all_names),
            out_names=tuple(out_names),
            lowering_input_output_aliases=(),
            sim_require_finite=True,
            sim_require_nnan=True,
            nc=nc,
        ))

    devices = jax.devices()[:NCORES]
    mesh = Mesh(np.asarray(devices), ("core",))
    in_specs = (PartitionSpec("core"),) * (n_params + n_outs)
    out_specs = (PartitionSpec("core"),) * n_outs
    donate_idx = tuple(range(n_params, n_params + n_outs)) if donate else ()
    sharded = jax.jit(
        shard_map(_body, mesh=mesh, in_specs=in_specs, out_specs=out_specs,
                  check_rep=False),
        donate_argnums=donate_idx, keep_unused=True,
    )

    _CACHE[("sharded", id(nc))] = sharded
    _CACHE[("zeros", id(nc))] = [((NCORES * z.shape[0],) + z.shape[1:], z.dtype)
                                 for z in zero_outs]

    def fn(in_maps):
        concat_in = [
            np.concatenate([np.asarray(in_maps[c][nm]) for c in range(NCORES)], axis=0)
            for nm in in_names
        ]
        concat_zeros = [
            np.zeros((NCORES * z.shape[0], *z.shape[1:]), z.dtype) for z in zero_outs
        ]
        out_arrs = sharded(*concat_in, *concat_zeros)
        return [
            {nm: np.asarray(out_arrs[i]).reshape(NCORES, *out_avals[i].shape)[c]
             for i, nm in enumerate(out_names)}
            for c in range(NCORES)
        ]

    _CACHE[key] = (fn, in_names, out_names)
    return _CACHE[key]


def _prep(inputs):
    x = np.asarray(inputs["x"], dtype=np.float32)
    qkv_b = np.asarray(inputs["qkv_b"], dtype=np.float32)
    has_v_bias = bool(np.any(qkv_b[512:768]))
    nc = _get_program(has_v_bias)
    in_maps = _make_in_maps(
        x,
        np.asarray(inputs["gn_w"], dtype=np.float32),
        np.asarray(inputs["gn_b"], dtype=np.float32),
        np.asarray(inputs["qkv_w"], dtype=np.float32),
        qkv_b,
        np.asarray(inputs["proj_w"], dtype=np.float32),
    )
    return nc, in_maps, x


def run(inputs, trace=False):
    """Run the sharded kernel.  Returns (output, per-core results list)."""
    nc, in_maps, x = _prep(inputs)
    fn, _, _ = _get_executor(nc)
    results = fn(in_maps)
    proj_b = np.asarray(inputs["proj_b"], dtype=np.float32)
    parts = [results[c]["yp"] for c in range(NCORES)]
    y = np.stack([parts[2 * b] + parts[2 * b + 1] for b in range(B)])  # [B, C, N]
    y = y + proj_b[None, :, None]
    out = np.asarray(inputs["x"], dtype=np.float32) + y.reshape(B, C, 64, 64)
    return out.astype(np.float32), results


def kernel(**inputs) -> np.ndarray:
    out, _ = run(inputs, trace=False)
    return out

